# revision 6
# baseline (speedup 1.0000x reference)
"""Trainium2 Bass kernel for BilinearGeneral:
out[b,k] = sum_ij x[b,i] W[k,i,j] z[b,j] + (z @ U.T)[b,k] + (x @ V.T)[b,k] + b[k]

Sharding: W/U/V/b split along OUT (tensor parallel) across 8 cores; x,z
replicated. Each core computes out[:, c*64:(c+1)*64]; host concatenates.

Per-core algorithm (KS=64 out features):
  prologue: UV[bt] = z@U_s.T + x@V_s.T + b_s          (small matmuls, PSUM)
  for k in range(64):
      T = x @ W_s[k]            # 4 f32r matmuls accumulated in PSUM [128b, 512j]
      out[:, k] = sum_j T*z + UV[:, k]   # ONE fused DVE tensor_tensor_reduce
"""

import numpy as np

B, IN1, IN2, OUT = 1024, 512, 512, 512
N_CORES = 8
KS = OUT // N_CORES  # 64 out features per core
P = 128
IC = IN1 // P  # 4 contraction chunks over i
JC = IN2 // P  # 4 contraction chunks over j
BT = B // P    # 8 batch tiles

TRACE = False
LAST_RESULTS = None

_compiled_nc = None


def _build():
    import concourse.tile as tile
    from concourse import bacc, mybir

    f32 = mybir.dt.float32
    f32r = mybir.dt.float32r
    AL = mybir.AluOpType

    nc = bacc.Bacc("TRN2", target_bir_lowering=False, debug=False,
                   num_devices=N_CORES)
    # Tensors feeding matmuls are declared float32r (same 4-byte layout as
    # f32) so the PE runs single-pass full-rate fp32r matmuls.
    xT_d = nc.dram_tensor("xT", [IN1, B], f32r, kind="ExternalInput").ap()
    zT_d = nc.dram_tensor("zT", [IN2, B], f32r, kind="ExternalInput").ap()
    z_d = nc.dram_tensor("z", [B, IN2], f32, kind="ExternalInput").ap()
    W_d = nc.dram_tensor("W", [KS, IN1, IN2], f32r, kind="ExternalInput").ap()
    UT_d = nc.dram_tensor("UT", [IN2, KS], f32r, kind="ExternalInput").ap()
    VT_d = nc.dram_tensor("VT", [IN1, KS], f32r, kind="ExternalInput").ap()
    b_d = nc.dram_tensor("bv", [1, KS], f32, kind="ExternalInput").ap()
    out_d = nc.dram_tensor("out", [B, KS], f32, kind="ExternalOutput").ap()

    with tile.TileContext(nc) as tc:
        with (
            tc.tile_pool(name="const", bufs=1) as cpool,
            tc.tile_pool(name="w", bufs=3) as wpool,
            tc.tile_pool(name="prod", bufs=4) as prodpool,
            tc.tile_pool(name="acc", bufs=1) as accpool,
            tc.tile_pool(name="ps", bufs=4, space="PSUM") as pspool,
            tc.tile_pool(name="psuv", bufs=2, space="PSUM") as psuvpool,
        ):
            xT_sb = cpool.tile([P, IC, B], f32r)
            nc.sync.dma_start(xT_sb[:], xT_d.rearrange("(ic p) b -> p ic b", p=P))
            zT_sb = cpool.tile([P, JC, B], f32r)
            nc.sync.dma_start(zT_sb[:], zT_d.rearrange("(jc p) b -> p jc b", p=P))
            z_sb = cpool.tile([P, BT, IN2], f32)
            nc.sync.dma_start(z_sb[:], z_d.rearrange("(bt p) j -> p bt j", p=P))
            UT_sb = cpool.tile([P, JC, KS], f32r)
            nc.sync.dma_start(UT_sb[:], UT_d.rearrange("(jc p) k -> p jc k", p=P))
            VT_sb = cpool.tile([P, IC, KS], f32r)
            nc.sync.dma_start(VT_sb[:], VT_d.rearrange("(ic p) k -> p ic k", p=P))
            b_sb = cpool.tile([1, KS], f32)
            nc.sync.dma_start(b_sb[:], b_d[:])
            ones_sb = cpool.tile([1, P], f32)
            nc.gpsimd.memset(ones_sb[:], 1.0)

            uv_sb = [accpool.tile([P, KS], f32, tag=f"uv{bt}", name=f"uv{bt}")
                     for bt in range(BT)]
            obt = [accpool.tile([P, KS], f32, tag=f"o{bt}", name=f"o{bt}")
                   for bt in range(BT)]

            # Prologue: UV[bt] = z@U_s.T + x@V_s.T + b_s (broadcast over rows)
            for bt in range(BT):
                pu = psuvpool.tile([P, KS], f32)
                for jc in range(JC):
                    nc.tensor.matmul(
                        pu[:], lhsT=zT_sb[:, jc, bt * P:(bt + 1) * P],
                        rhs=UT_sb[:, jc, :], start=(jc == 0), stop=False)
                for ic in range(IC):
                    nc.tensor.matmul(
                        pu[:], lhsT=xT_sb[:, ic, bt * P:(bt + 1) * P],
                        rhs=VT_sb[:, ic, :], start=False, stop=False)
                nc.tensor.matmul(pu[:], lhsT=ones_sb[:, :], rhs=b_sb[:, :],
                                 start=False, stop=True)
                nc.scalar.copy(uv_sb[bt][:], pu[:])

            # Main loop over this core's out features
            for k in range(KS):
                wk = wpool.tile([P, IC, IN2], f32r)
                nc.sync.dma_start(wk[:], W_d[k].rearrange("(ic p) j -> p ic j", p=P))
                for bt in range(BT):
                    ps = pspool.tile([P, IN2], f32)
                    for ic in range(IC):
                        nc.tensor.matmul(
                            ps[:],
                            lhsT=xT_sb[:, ic, bt * P:(bt + 1) * P],
                            rhs=wk[:, ic, :],
                            start=(ic == 0), stop=(ic == IC - 1))
                    prod = prodpool.tile([P, IN2], f32)
                    nc.vector.scalar_tensor_tensor(
                        out=prod[:],
                        in0=ps[:],
                        scalar=0.0,
                        in1=z_sb[:, bt, :],
                        op0=AL.bypass,
                        op1=AL.mult,
                        accum_out=obt[bt][:, k:k + 1])

            for bt in range(BT):
                nc.vector.tensor_add(obt[bt][:], obt[bt][:], uv_sb[bt][:])
                nc.sync.dma_start(out_d[bt * P:(bt + 1) * P, :], obt[bt][:])

    nc.compile()
    return nc


def kernel(x, z, W, U, V, b):
    global _compiled_nc, LAST_RESULTS
    from concourse.bass_utils import run_bass_kernel_spmd

    x = np.asarray(x, dtype=np.float32)
    z = np.asarray(z, dtype=np.float32)
    W = np.asarray(W, dtype=np.float32)
    U = np.asarray(U, dtype=np.float32)
    V = np.asarray(V, dtype=np.float32)
    b = np.asarray(b, dtype=np.float32)

    if _compiled_nc is None:
        _compiled_nc = _build()
    nc = _compiled_nc

    xT = np.ascontiguousarray(x.T)
    zT = np.ascontiguousarray(z.T)
    in_maps = []
    for c in range(N_CORES):
        k0, k1 = c * KS, (c + 1) * KS
        in_maps.append({
            "xT": xT,
            "zT": zT,
            "z": z,
            "W": W[k0:k1],
            "UT": np.ascontiguousarray(U[k0:k1].T),
            "VT": np.ascontiguousarray(V[k0:k1].T),
            "bv": np.ascontiguousarray(b[k0:k1].reshape(1, KS)),
        })

    res = run_bass_kernel_spmd(
        nc, in_maps, core_ids=list(range(N_CORES)), trace=TRACE,
        trace_cores=[0] if TRACE else None)
    LAST_RESULTS = res
    out = np.concatenate([res.results[c]["out"] for c in range(N_CORES)], axis=1)
    return out


# revision 7
# speedup vs baseline: 1.0087x; 1.0087x over previous
"""Trainium2 Bass kernel for BilinearGeneral:
out[b,k] = sum_ij x[b,i] W[k,i,j] z[b,j] + (z @ U.T)[b,k] + (x @ V.T)[b,k] + b[k]

Sharding: W/U/V/b split along OUT (tensor parallel) across 8 cores; x,z
replicated. Each core computes out[:, c*64:(c+1)*64]; host concatenates.

Per-core algorithm (KS=64 out features):
  prologue: UV[bt] = z@U_s.T + x@V_s.T + b_s          (small matmuls, PSUM)
  for k in range(64):
      T = x @ W_s[k]            # 4 f32r matmuls accumulated in PSUM [128b, 512j]
      out[:, k] = sum_j T*z + UV[:, k]   # ONE fused DVE tensor_tensor_reduce
"""

import numpy as np

B, IN1, IN2, OUT = 1024, 512, 512, 512
N_CORES = 8
KS = OUT // N_CORES  # 64 out features per core
P = 128
IC = IN1 // P  # 4 contraction chunks over i
JC = IN2 // P  # 4 contraction chunks over j
BT = B // P    # 8 batch tiles

TRACE = False
LAST_RESULTS = None

_compiled_nc = None


def _build():
    import concourse.tile as tile
    from concourse import bacc, mybir

    f32 = mybir.dt.float32
    f32r = mybir.dt.float32r
    AL = mybir.AluOpType

    nc = bacc.Bacc("TRN2", target_bir_lowering=False, debug=False,
                   num_devices=N_CORES)
    # Tensors feeding matmuls are declared float32r (same 4-byte layout as
    # f32) so the PE runs single-pass full-rate fp32r matmuls.
    xT_d = nc.dram_tensor("xT", [IN1, B], f32r, kind="ExternalInput").ap()
    zT_d = nc.dram_tensor("zT", [IN2, B], f32r, kind="ExternalInput").ap()
    z_d = nc.dram_tensor("z", [B, IN2], f32, kind="ExternalInput").ap()
    W_d = nc.dram_tensor("W", [KS, IN1, IN2], f32r, kind="ExternalInput").ap()
    UT_d = nc.dram_tensor("UT", [IN2, KS], f32r, kind="ExternalInput").ap()
    VT_d = nc.dram_tensor("VT", [IN1, KS], f32r, kind="ExternalInput").ap()
    b_d = nc.dram_tensor("bv", [1, KS], f32, kind="ExternalInput").ap()
    out_d = nc.dram_tensor("out", [B, KS], f32, kind="ExternalOutput").ap()

    with tile.TileContext(nc) as tc:
        with (
            tc.tile_pool(name="const", bufs=1) as cpool,
            tc.tile_pool(name="w", bufs=4) as wpool,
            tc.tile_pool(name="prod", bufs=4) as prodpool,
            tc.tile_pool(name="acc", bufs=1) as accpool,
            tc.tile_pool(name="ps", bufs=6, space="PSUM") as pspool,
            tc.tile_pool(name="psuv", bufs=2, space="PSUM") as psuvpool,
        ):
            # Critical-path inputs first: xT (stationary operands) and the
            # first W tiles gate the first matmul; z gates the first DVE op.
            xT_sb = cpool.tile([P, IC, B], f32r)
            for ic in range(IC):
                nc.sync.dma_start(xT_sb[:, ic, :], xT_d[ic * P:(ic + 1) * P, :])
            z_sb = cpool.tile([P, BT, IN2], f32)
            nc.sync.dma_start(z_sb[:], z_d.rearrange("(bt p) j -> p bt j", p=P))

            obt = [accpool.tile([P, KS], f32, tag=f"o{bt}", name=f"o{bt}")
                   for bt in range(BT)]

            # Main loop over this core's out features
            for k in range(KS):
                wk = wpool.tile([P, IC, IN2], f32r)
                wv = W_d[k].rearrange("(ic p) j -> p ic j", p=P)
                nc.sync.dma_start(wk[:, 0:2, :], wv[:, 0:2, :])
                nc.sync.dma_start(wk[:, 2:4, :], wv[:, 2:4, :])
                for bt in range(BT):
                    ps = pspool.tile([P, IN2], f32)
                    for ic in range(IC):
                        nc.tensor.matmul(
                            ps[:],
                            lhsT=xT_sb[:, ic, bt * P:(bt + 1) * P],
                            rhs=wk[:, ic, :],
                            start=(ic == 0), stop=(ic == IC - 1))
                    prod = prodpool.tile([P, IN2], f32)
                    nc.vector.scalar_tensor_tensor(
                        out=prod[:],
                        in0=ps[:],
                        scalar=0.0,
                        in1=z_sb[:, bt, :],
                        op0=AL.bypass,
                        op1=AL.mult,
                        accum_out=obt[bt][:, k:k + 1])

            # UV block (off the critical path — DMAs fill idle DMA slots
            # mid-kernel; matmuls run in the tail): UV[bt] = z@U_s.T +
            # x@V_s.T + b_s (broadcast over rows)
            zT_sb = cpool.tile([P, JC, B], f32r)
            nc.sync.dma_start(zT_sb[:], zT_d.rearrange("(jc p) b -> p jc b", p=P))
            UT_sb = cpool.tile([P, JC, KS], f32r)
            nc.sync.dma_start(UT_sb[:], UT_d.rearrange("(jc p) k -> p jc k", p=P))
            VT_sb = cpool.tile([P, IC, KS], f32r)
            nc.sync.dma_start(VT_sb[:], VT_d.rearrange("(ic p) k -> p ic k", p=P))
            b_sb = cpool.tile([1, KS], f32)
            nc.sync.dma_start(b_sb[:], b_d[:])
            ones_sb = cpool.tile([1, P], f32)
            nc.gpsimd.memset(ones_sb[:], 1.0)

            uv_sb = [accpool.tile([P, KS], f32, tag=f"uv{bt}", name=f"uv{bt}")
                     for bt in range(BT)]
            for bt in range(BT):
                pu = psuvpool.tile([P, KS], f32)
                for jc in range(JC):
                    nc.tensor.matmul(
                        pu[:], lhsT=zT_sb[:, jc, bt * P:(bt + 1) * P],
                        rhs=UT_sb[:, jc, :], start=(jc == 0), stop=False)
                for ic in range(IC):
                    nc.tensor.matmul(
                        pu[:], lhsT=xT_sb[:, ic, bt * P:(bt + 1) * P],
                        rhs=VT_sb[:, ic, :], start=False, stop=False)
                nc.tensor.matmul(pu[:], lhsT=ones_sb[:, :], rhs=b_sb[:, :],
                                 start=False, stop=True)
                nc.scalar.copy(uv_sb[bt][:], pu[:])

            for bt in range(BT):
                nc.vector.tensor_add(obt[bt][:], obt[bt][:], uv_sb[bt][:])
                nc.sync.dma_start(out_d[bt * P:(bt + 1) * P, :], obt[bt][:])

    nc.compile()
    return nc


def kernel(x, z, W, U, V, b):
    global _compiled_nc, LAST_RESULTS
    from concourse.bass_utils import run_bass_kernel_spmd

    x = np.asarray(x, dtype=np.float32)
    z = np.asarray(z, dtype=np.float32)
    W = np.asarray(W, dtype=np.float32)
    U = np.asarray(U, dtype=np.float32)
    V = np.asarray(V, dtype=np.float32)
    b = np.asarray(b, dtype=np.float32)

    if _compiled_nc is None:
        _compiled_nc = _build()
    nc = _compiled_nc

    xT = np.ascontiguousarray(x.T)
    zT = np.ascontiguousarray(z.T)
    in_maps = []
    for c in range(N_CORES):
        k0, k1 = c * KS, (c + 1) * KS
        in_maps.append({
            "xT": xT,
            "zT": zT,
            "z": z,
            "W": W[k0:k1],
            "UT": np.ascontiguousarray(U[k0:k1].T),
            "VT": np.ascontiguousarray(V[k0:k1].T),
            "bv": np.ascontiguousarray(b[k0:k1].reshape(1, KS)),
        })

    res = run_bass_kernel_spmd(
        nc, in_maps, core_ids=list(range(N_CORES)), trace=TRACE,
        trace_cores=[0] if TRACE else None)
    LAST_RESULTS = res
    out = np.concatenate([res.results[c]["out"] for c in range(N_CORES)], axis=1)
    return out


# revision 8
# speedup vs baseline: 1.0169x; 1.0081x over previous
"""Trainium2 Bass kernel for BilinearGeneral:
out[b,k] = sum_ij x[b,i] W[k,i,j] z[b,j] + (z @ U.T)[b,k] + (x @ V.T)[b,k] + b[k]

Sharding: W/U/V/b split along OUT (tensor parallel) across 8 cores; x,z
replicated. Each core computes out[:, c*64:(c+1)*64]; host concatenates.

Per-core algorithm (KS=64 out features):
  prologue: UV[bt] = z@U_s.T + x@V_s.T + b_s          (small matmuls, PSUM)
  for k in range(64):
      T = x @ W_s[k]            # 4 f32r matmuls accumulated in PSUM [128b, 512j]
      out[:, k] = sum_j T*z + UV[:, k]   # ONE fused DVE tensor_tensor_reduce
"""

import numpy as np

B, IN1, IN2, OUT = 1024, 512, 512, 512
N_CORES = 8
KS = OUT // N_CORES  # 64 out features per core
P = 128
IC = IN1 // P  # 4 contraction chunks over i
JC = IN2 // P  # 4 contraction chunks over j
BT = B // P    # 8 batch tiles

TRACE = False
LAST_RESULTS = None

_compiled_nc = None


def _build():
    import concourse.tile as tile
    from concourse import bacc, mybir

    f32 = mybir.dt.float32
    f32r = mybir.dt.float32r
    AL = mybir.AluOpType

    nc = bacc.Bacc("TRN2", target_bir_lowering=False, debug=False,
                   num_devices=N_CORES)
    # Tensors feeding matmuls are declared float32r (same 4-byte layout as
    # f32) so the PE runs single-pass full-rate fp32r matmuls.
    xT_d = nc.dram_tensor("xT", [IN1, B], f32r, kind="ExternalInput").ap()
    zT_d = nc.dram_tensor("zT", [IN2, B], f32r, kind="ExternalInput").ap()
    z_d = nc.dram_tensor("z", [B, IN2], f32, kind="ExternalInput").ap()
    W_d = nc.dram_tensor("W", [KS, IN1, IN2], f32r, kind="ExternalInput").ap()
    UT_d = nc.dram_tensor("UT", [IN2, KS], f32r, kind="ExternalInput").ap()
    VT_d = nc.dram_tensor("VT", [IN1, KS], f32r, kind="ExternalInput").ap()
    b_d = nc.dram_tensor("bv", [1, KS], f32, kind="ExternalInput").ap()
    out_d = nc.dram_tensor("out", [B, KS], f32, kind="ExternalOutput").ap()

    with tile.TileContext(nc) as tc:
        with (
            tc.tile_pool(name="const", bufs=1) as cpool,
            tc.tile_pool(name="w", bufs=4) as wpool,
            tc.tile_pool(name="prod", bufs=4) as prodpool,
            tc.tile_pool(name="acc", bufs=1) as accpool,
            tc.tile_pool(name="ps", bufs=6, space="PSUM") as pspool,
            tc.tile_pool(name="psuv", bufs=2, space="PSUM") as psuvpool,
        ):
            # Critical-path inputs first: xT (stationary operands) and the
            # first W tiles gate the first matmul; z gates the first DVE op.
            xT_sb = cpool.tile([P, IC, B], f32r)
            for ic in range(IC):
                nc.sync.dma_start(xT_sb[:, ic, :], xT_d[ic * P:(ic + 1) * P, :])
            # z goes on the ACT-engine HWDGE queue so it doesn't serialize
            # behind xT/W[0] on the sync-engine queue (z gates only the DVE).
            z_sb = cpool.tile([P, BT, IN2], f32)
            zv = z_d.rearrange("(bt p) j -> p bt j", p=P)
            nc.scalar.dma_start(z_sb[:, 0:1, :], zv[:, 0:1, :])
            nc.scalar.dma_start(z_sb[:, 1:BT, :], zv[:, 1:BT, :])

            obt = [accpool.tile([P, KS], f32, tag=f"o{bt}", name=f"o{bt}")
                   for bt in range(BT)]

            # Main loop over this core's out features
            for k in range(KS):
                wk = wpool.tile([P, IC, IN2], f32r)
                wv = W_d[k].rearrange("(ic p) j -> p ic j", p=P)
                nc.sync.dma_start(wk[:, 0:2, :], wv[:, 0:2, :])
                nc.sync.dma_start(wk[:, 2:4, :], wv[:, 2:4, :])
                for bt in range(BT):
                    ps = pspool.tile([P, IN2], f32)
                    for ic in range(IC):
                        nc.tensor.matmul(
                            ps[:],
                            lhsT=xT_sb[:, ic, bt * P:(bt + 1) * P],
                            rhs=wk[:, ic, :],
                            start=(ic == 0), stop=(ic == IC - 1))
                    prod = prodpool.tile([P, IN2], f32)
                    nc.vector.scalar_tensor_tensor(
                        out=prod[:],
                        in0=ps[:],
                        scalar=0.0,
                        in1=z_sb[:, bt, :],
                        op0=AL.bypass,
                        op1=AL.mult,
                        accum_out=obt[bt][:, k:k + 1])

            # UV block (off the critical path — DMAs fill idle DMA slots
            # mid-kernel; matmuls run in the tail): UV[bt] = z@U_s.T +
            # x@V_s.T + b_s (broadcast over rows)
            zT_sb = cpool.tile([P, JC, B], f32r)
            nc.sync.dma_start(zT_sb[:], zT_d.rearrange("(jc p) b -> p jc b", p=P))
            UT_sb = cpool.tile([P, JC, KS], f32r)
            nc.sync.dma_start(UT_sb[:], UT_d.rearrange("(jc p) k -> p jc k", p=P))
            VT_sb = cpool.tile([P, IC, KS], f32r)
            nc.sync.dma_start(VT_sb[:], VT_d.rearrange("(ic p) k -> p ic k", p=P))
            b_sb = cpool.tile([1, KS], f32)
            nc.sync.dma_start(b_sb[:], b_d[:])
            ones_sb = cpool.tile([1, P], f32)
            nc.gpsimd.memset(ones_sb[:], 1.0)

            uv_sb = [accpool.tile([P, KS], f32, tag=f"uv{bt}", name=f"uv{bt}")
                     for bt in range(BT)]
            for bt in range(BT):
                pu = psuvpool.tile([P, KS], f32)
                for jc in range(JC):
                    nc.tensor.matmul(
                        pu[:], lhsT=zT_sb[:, jc, bt * P:(bt + 1) * P],
                        rhs=UT_sb[:, jc, :], start=(jc == 0), stop=False)
                for ic in range(IC):
                    nc.tensor.matmul(
                        pu[:], lhsT=xT_sb[:, ic, bt * P:(bt + 1) * P],
                        rhs=VT_sb[:, ic, :], start=False, stop=False)
                nc.tensor.matmul(pu[:], lhsT=ones_sb[:, :], rhs=b_sb[:, :],
                                 start=False, stop=True)
                nc.scalar.copy(uv_sb[bt][:], pu[:])

            for bt in range(BT):
                nc.vector.tensor_add(obt[bt][:], obt[bt][:], uv_sb[bt][:])
                nc.sync.dma_start(out_d[bt * P:(bt + 1) * P, :], obt[bt][:])

    nc.compile()
    return nc


def kernel(x, z, W, U, V, b):
    global _compiled_nc, LAST_RESULTS
    from concourse.bass_utils import run_bass_kernel_spmd

    x = np.asarray(x, dtype=np.float32)
    z = np.asarray(z, dtype=np.float32)
    W = np.asarray(W, dtype=np.float32)
    U = np.asarray(U, dtype=np.float32)
    V = np.asarray(V, dtype=np.float32)
    b = np.asarray(b, dtype=np.float32)

    if _compiled_nc is None:
        _compiled_nc = _build()
    nc = _compiled_nc

    xT = np.ascontiguousarray(x.T)
    zT = np.ascontiguousarray(z.T)
    in_maps = []
    for c in range(N_CORES):
        k0, k1 = c * KS, (c + 1) * KS
        in_maps.append({
            "xT": xT,
            "zT": zT,
            "z": z,
            "W": W[k0:k1],
            "UT": np.ascontiguousarray(U[k0:k1].T),
            "VT": np.ascontiguousarray(V[k0:k1].T),
            "bv": np.ascontiguousarray(b[k0:k1].reshape(1, KS)),
        })

    res = run_bass_kernel_spmd(
        nc, in_maps, core_ids=list(range(N_CORES)), trace=TRACE,
        trace_cores=[0] if TRACE else None)
    LAST_RESULTS = res
    out = np.concatenate([res.results[c]["out"] for c in range(N_CORES)], axis=1)
    return out


# revision 10
# speedup vs baseline: 1.0287x; 1.0117x over previous
"""Trainium2 Bass kernel for BilinearGeneral:
out[b,k] = sum_ij x[b,i] W[k,i,j] z[b,j] + (z @ U.T)[b,k] + (x @ V.T)[b,k] + b[k]

Sharding: W/U/V/b split along OUT (tensor parallel) across 8 cores; x,z
replicated. Each core computes out[:, c*64:(c+1)*64]; host concatenates.

Per-core algorithm (KS=64 out features):
  prologue: UV[bt] = z@U_s.T + x@V_s.T + b_s          (small matmuls, PSUM)
  for k in range(64):
      T = x @ W_s[k]            # 4 f32r matmuls accumulated in PSUM [128b, 512j]
      out[:, k] = sum_j T*z + UV[:, k]   # ONE fused DVE tensor_tensor_reduce
"""

import numpy as np

B, IN1, IN2, OUT = 1024, 512, 512, 512
N_CORES = 8
KS = OUT // N_CORES  # 64 out features per core
P = 128
IC = IN1 // P  # 4 contraction chunks over i
JC = IN2 // P  # 4 contraction chunks over j
BT = B // P    # 8 batch tiles

TRACE = False
LAST_RESULTS = None

_compiled_nc = None


def _build():
    import concourse.tile as tile
    from concourse import bacc, mybir

    f32 = mybir.dt.float32
    f32r = mybir.dt.float32r
    AL = mybir.AluOpType

    nc = bacc.Bacc("TRN2", target_bir_lowering=False, debug=False,
                   num_devices=N_CORES)
    # Tensors feeding matmuls are declared float32r (same 4-byte layout as
    # f32) so the PE runs single-pass full-rate fp32r matmuls.
    xT_d = nc.dram_tensor("xT", [IN1, B], f32r, kind="ExternalInput").ap()
    zT_d = nc.dram_tensor("zT", [IN2, B], f32r, kind="ExternalInput").ap()
    z_d = nc.dram_tensor("z", [B, IN2], f32, kind="ExternalInput").ap()
    W_d = nc.dram_tensor("W", [KS, IN1, IN2], f32r, kind="ExternalInput").ap()
    UT_d = nc.dram_tensor("UT", [IN2, KS], f32r, kind="ExternalInput").ap()
    VT_d = nc.dram_tensor("VT", [IN1, KS], f32r, kind="ExternalInput").ap()
    b_d = nc.dram_tensor("bv", [1, KS], f32, kind="ExternalInput").ap()
    out_d = nc.dram_tensor("out", [B, KS], f32, kind="ExternalOutput").ap()

    with tile.TileContext(nc) as tc:
        with (
            tc.tile_pool(name="const", bufs=1) as cpool,
            tc.tile_pool(name="w", bufs=4) as wpool,
            tc.tile_pool(name="prod", bufs=4) as prodpool,
            tc.tile_pool(name="acc", bufs=1) as accpool,
            tc.tile_pool(name="ps", bufs=6, space="PSUM") as pspool,
            tc.tile_pool(name="psuv", bufs=2, space="PSUM") as psuvpool,
        ):
            # Critical-path inputs first: xT (stationary operands) and the
            # first W tiles gate the first matmul; z gates the first DVE op.
            xT_sb = cpool.tile([P, IC, B], f32r)
            for ic in range(IC):
                nc.sync.dma_start(xT_sb[:, ic, :], xT_d[ic * P:(ic + 1) * P, :])
            # Queue order on the sync HWDGE queue: xT, W[0], z, W[1], ...
            # so the matmul critical path (xT + W[0]) streams first and z
            # (which only gates the DVE ops) follows right behind.
            z_sb = cpool.tile([P, BT, IN2], f32)
            zv = z_d.rearrange("(bt p) j -> p bt j", p=P)

            def load_wk(k):
                wk = wpool.tile([P, IC, IN2], f32r, tag="wk", name=f"wk{k}")
                wv = W_d[k].rearrange("(ic p) j -> p ic j", p=P)
                nc.sync.dma_start(wk[:, 0:2, :], wv[:, 0:2, :])
                nc.sync.dma_start(wk[:, 2:4, :], wv[:, 2:4, :])
                return wk

            wk0 = load_wk(0)
            nc.sync.dma_start(z_sb[:, 0:2, :], zv[:, 0:2, :])
            nc.sync.dma_start(z_sb[:, 2:BT, :], zv[:, 2:BT, :])

            obt = [accpool.tile([P, KS], f32, tag=f"o{bt}", name=f"o{bt}")
                   for bt in range(BT)]

            # Main loop over this core's out features
            for k in range(KS):
                wk = wk0 if k == 0 else load_wk(k)
                for bt in range(BT):
                    ps = pspool.tile([P, IN2], f32)
                    for ic in range(IC):
                        nc.tensor.matmul(
                            ps[:],
                            lhsT=xT_sb[:, ic, bt * P:(bt + 1) * P],
                            rhs=wk[:, ic, :],
                            start=(ic == 0), stop=(ic == IC - 1))
                    prod = prodpool.tile([P, IN2], f32)
                    nc.vector.scalar_tensor_tensor(
                        out=prod[:],
                        in0=ps[:],
                        scalar=0.0,
                        in1=z_sb[:, bt, :],
                        op0=AL.bypass,
                        op1=AL.mult,
                        accum_out=obt[bt][:, k:k + 1])

            # UV block (off the critical path — DMAs fill idle DMA slots
            # mid-kernel; matmuls run in the tail): UV[bt] = z@U_s.T +
            # x@V_s.T + b_s (broadcast over rows)
            zT_sb = cpool.tile([P, JC, B], f32r)
            nc.sync.dma_start(zT_sb[:], zT_d.rearrange("(jc p) b -> p jc b", p=P))
            UT_sb = cpool.tile([P, JC, KS], f32r)
            nc.sync.dma_start(UT_sb[:], UT_d.rearrange("(jc p) k -> p jc k", p=P))
            VT_sb = cpool.tile([P, IC, KS], f32r)
            nc.sync.dma_start(VT_sb[:], VT_d.rearrange("(ic p) k -> p ic k", p=P))
            b_sb = cpool.tile([1, KS], f32)
            nc.sync.dma_start(b_sb[:], b_d[:])
            ones_sb = cpool.tile([1, P], f32)
            nc.gpsimd.memset(ones_sb[:], 1.0)

            uv_sb = [accpool.tile([P, KS], f32, tag=f"uv{bt}", name=f"uv{bt}")
                     for bt in range(BT)]
            for bt in range(BT):
                pu = psuvpool.tile([P, KS], f32)
                for jc in range(JC):
                    nc.tensor.matmul(
                        pu[:], lhsT=zT_sb[:, jc, bt * P:(bt + 1) * P],
                        rhs=UT_sb[:, jc, :], start=(jc == 0), stop=False)
                for ic in range(IC):
                    nc.tensor.matmul(
                        pu[:], lhsT=xT_sb[:, ic, bt * P:(bt + 1) * P],
                        rhs=VT_sb[:, ic, :], start=False, stop=False)
                nc.tensor.matmul(pu[:], lhsT=ones_sb[:, :], rhs=b_sb[:, :],
                                 start=False, stop=True)
                nc.scalar.copy(uv_sb[bt][:], pu[:])

            for bt in range(BT):
                nc.vector.tensor_add(obt[bt][:], obt[bt][:], uv_sb[bt][:])
                nc.sync.dma_start(out_d[bt * P:(bt + 1) * P, :], obt[bt][:])

    nc.compile()
    return nc


def kernel(x, z, W, U, V, b):
    global _compiled_nc, LAST_RESULTS
    from concourse.bass_utils import run_bass_kernel_spmd

    x = np.asarray(x, dtype=np.float32)
    z = np.asarray(z, dtype=np.float32)
    W = np.asarray(W, dtype=np.float32)
    U = np.asarray(U, dtype=np.float32)
    V = np.asarray(V, dtype=np.float32)
    b = np.asarray(b, dtype=np.float32)

    if _compiled_nc is None:
        _compiled_nc = _build()
    nc = _compiled_nc

    xT = np.ascontiguousarray(x.T)
    zT = np.ascontiguousarray(z.T)
    in_maps = []
    for c in range(N_CORES):
        k0, k1 = c * KS, (c + 1) * KS
        in_maps.append({
            "xT": xT,
            "zT": zT,
            "z": z,
            "W": W[k0:k1],
            "UT": np.ascontiguousarray(U[k0:k1].T),
            "VT": np.ascontiguousarray(V[k0:k1].T),
            "bv": np.ascontiguousarray(b[k0:k1].reshape(1, KS)),
        })

    res = run_bass_kernel_spmd(
        nc, in_maps, core_ids=list(range(N_CORES)), trace=TRACE,
        trace_cores=[0] if TRACE else None)
    LAST_RESULTS = res
    out = np.concatenate([res.results[c]["out"] for c in range(N_CORES)], axis=1)
    return out
